# revision 17
# baseline (speedup 1.0000x reference)
"""Causal self-attention (softmax over the QUERY axis) for Trainium2, 8 cores.

Reference semantics (B=2, S=2048, D=1024, H=16, HD=64):
    q = x @ Wq; k = x @ Wk; v = x @ Wv          (per batch)
    s[b,h,q,k] = <q_bqh, k_bkh>;  mask k > q -> -inf
    w = softmax(s / sqrt(1024), axis=q)          # normalize over QUERY axis
    ctx[b,q,h,:] = sum_k w[b,h,q,k] * v[b,k,h,:]

Sharding: core c handles batch b = c // 4 and head group g = c % 4
(4 heads: 4g..4g+3).  Per core everything is done in a transposed
score layout S^T[k, q]: the query-axis softmax is a FREE-AXIS
reduction, and the 1/Z[k] normalizer folds into V rows:
ctx[q,d] = sum_k exp(s)/Z[k] * v[k,d] = sum_k exp(s) * (v[k,d]/Z[k]).

Score matmuls run in fp8e4 DoubleRow perf mode (0.5 PE cycles/col):
the two contraction sub-rows hold k_hi=fp8(k) and k_lo=fp8(k-k_hi), so
k is effectively bf16-accurate and only q carries fp8 quantization
error (~0.8% end-to-end, tolerance 2e-2). q is fed with a stride-0
duplicated sub-row AP. Projections and ctx matmuls stay bf16.

Per head the 16 causal score rows (row kt: [128 keys, 2048-128*kt q])
are packed into 14 psum "bins" (<=1536 f32 cols = 3 banks); each bin
is ONE activation instruction (exp, psum->SBUF bf16 into a contiguous
per-head E tile). Diagonal blocks are zeroed post-exp (gpsimd
affine_select); Z[k] row sums are reduced post-mask (DVE rows 0-3,
Pool rows 4-15) so no separate invalid-sum correction is needed.
"""

import numpy as np
import ml_dtypes
from contextlib import ExitStack

import concourse.bass as bass
import concourse.tile as tile
from concourse import bacc, mybir
from concourse.bass_utils import run_bass_kernel_spmd

BF16 = mybir.dt.bfloat16
F8 = mybir.dt.float8e4
F32 = mybir.dt.float32

B, S, D, H, HD = 2, 2048, 1024, 16, 64
NCORES = 8
HL = 4                       # heads per core
KC = D // 128                # 8 contraction chunks
KT = S // 128                # 16 key tiles
SCALE = 1.0 / float(np.sqrt(np.float32(D)))   # 1/32
WS = 16.0                                     # host pre-scale on Wq/Wk
SCALE_SC = SCALE / (WS * WS)                  # exp scale for scaled scores

# Score-row bins: each bin = one <=1536-col psum tile = one exp instr.
# Pieces are (kt, lo, hi) in row-local query cols (row kt spans
# global q [128*kt, 2048), width 2048-128*kt).
BINS = [
    [(0, 0, 1536)],                           # b0
    [(0, 1536, 2048), (1, 1536, 1920)],       # b1   512+384
    [(1, 0, 1536)],                           # b2
    [(2, 0, 1536)],                           # b3
    [(2, 1536, 1792), (3, 1536, 1664)],       # b4   256+128
    [(3, 0, 1536)],                           # b5
    [(4, 0, 1536)],                           # b6
    [(5, 0, 1408)],                           # b7
    [(6, 0, 1280)],                           # b8
    [(7, 0, 1152)],                           # b9
    [(8, 0, 1024), (14, 0, 256), (15, 0, 128)],  # b10  1408
    [(10, 0, 768), (11, 0, 640)],             # b11  1408
    [(9, 0, 896), (13, 0, 384)],              # b12  1280
    [(12, 0, 512)],                           # b13
]
BIN_W = [sum(hi - lo for _, lo, hi in b) for b in BINS]
BIN_OFF = np.concatenate([[0], np.cumsum(BIN_W)]).astype(int)
E_TOT = int(BIN_OFF[-1])                      # 17408

# E segments per row: kt -> list of (lo, hi, absolute E-tile offset)
SEG = {kt: [] for kt in range(KT)}
for bi, pieces in enumerate(BINS):
    off = int(BIN_OFF[bi])
    for kt, lo, hi in pieces:
        SEG[kt].append((lo, hi, off))
        off += hi - lo
for kt in range(KT):
    SEG[kt].sort()

# single-row bins whose Z comes from the activation accumulator
ACCUM_BINS = {0: 0, 2: 1, 3: 2, 5: 3, 6: 4, 7: 5, 8: 6, 9: 7, 13: 12}

# bins ready after projection chain qc (chains emitted 3,2,1,0):
# ready class = min over rows of kt//4
AVAIL = [[13], [10, 11, 12], [6, 7, 8, 9], [0, 1, 2, 3, 4, 5]]
# head-3 progressive groups: after group g, rows 4g..4g+3 are complete
H3_GROUPS = [[0, 1, 2, 3, 4, 5], [6, 7, 8, 9], [10, 11, 12], [13]]


def _emit(ctx: ExitStack, tc: tile.TileContext, out_ap, xT, x8, wq, wk, wv):
    nc = tc.nc
    Exp = mybir.ActivationFunctionType.Exp

    consts = ctx.enter_context(tc.tile_pool(name="consts", bufs=1))
    qkp = ctx.enter_context(tc.tile_pool(name="qk", bufs=1))
    vp = ctx.enter_context(tc.tile_pool(name="v", bufs=1))
    epool = ctx.enter_context(tc.tile_pool(name="e", bufs=3))
    zpool = ctx.enter_context(tc.tile_pool(name="z", bufs=4))
    spool = ctx.enter_context(tc.tile_pool(name="scr", bufs=4))
    outp = ctx.enter_context(tc.tile_pool(name="outp", bufs=1))
    sc_ps = ctx.enter_context(tc.tile_pool(name="sc_ps", bufs=2, space="PSUM"))
    small_ps = ctx.enter_context(tc.tile_pool(name="small_ps", bufs=2, space="PSUM"))

    # ---- input DMAs spread over the two HWDGE rings, critical-first ----
    w8_sb = {}
    for name, t, eng in (("q", wq, nc.sync), ("k", wk, nc.scalar)):
        w8_sb[name] = consts.tile([128, KC, 2, HL * HD], F8, tag=f"w{name}8",
                                  name=f"w{name}8_sb")
        eng.dma_start(out=w8_sb[name],
                      in_=t.rearrange("(c p) two n -> p c two n", p=128))
    x8_r = x8.rearrange("(c p) s -> p c s", p=128)
    x8_cs = [None] * 4
    for sc in range(4):
        x8_cs[sc] = consts.tile([128, KC, 512], F8, tag=f"x8{sc}",
                                name=f"x8{sc}_sb")
    # enqueue cost is ~0.6us per dma_start, so few DMAs, critical first
    nc.sync.dma_start(out=x8_cs[3], in_=x8_r[:, :, 1536:2048])
    nc.scalar.dma_start(out=x8_cs[2], in_=x8_r[:, :, 1024:1536])
    nc.sync.dma_start(out=x8_cs[1], in_=x8_r[:, :, 512:1024])
    nc.scalar.dma_start(out=x8_cs[0], in_=x8_r[:, :, 0:512])
    w_sb = {}
    w_sb["v"] = consts.tile([128, KC, HL * HD], BF16, tag="wv", name="wv_sb")
    nc.scalar.dma_start(out=w_sb["v"],
                        in_=wv.rearrange("(c p) n -> p c n", p=128))
    xT_r = xT.rearrange("(c p) s -> p c s", p=128)
    xT_cs = [None] * 4
    for sc in range(4):
        xT_cs[sc] = consts.tile([128, KC, 512], BF16, tag=f"xT{sc}",
                                name=f"xT{sc}_sb")
    # v-proj consumes chunk 0 first
    nc.sync.dma_start(out=xT_cs[0], in_=xT_r[:, :, 0:512])
    nc.scalar.dma_start(out=xT_cs[1], in_=xT_r[:, :, 512:1024])
    nc.sync.dma_start(out=xT_cs[2], in_=xT_r[:, :, 1024:1536])
    nc.scalar.dma_start(out=xT_cs[3], in_=xT_r[:, :, 1536:2048])

    def xT_slice(c, lo, w):
        sc, o = divmod(lo, 512)
        assert o + w <= 512
        return xT_cs[sc][:, c, o:o + w]

    # q: fp8, duplicated sub-rows via stride-0 AP at use site
    qf8 = qkp.tile([128, 2, S], F8, tag="qf8")
    # k: fp8 hi/lo residual sub-rows
    kf8 = qkp.tile([128, 2, 2, S], F8, tag="kf8")
    v_sb = vp.tile([128, KT, HL * HD], BF16, tag="v")
    v2_sb = vp.tile([128, KT, HL * HD], BF16, tag="v2")

    def proj_chain(name, pair, qc_pair):
        tiles = [small_ps.tile([128, 512], F32, tag="ps512", name=f"pp{i}")
                 for i in range(2)]
        for c in range(KC):
            lhsT = w8_sb[name][:, c, :, 128 * pair:128 * pair + 128]
            for ps, qc in zip(tiles, qc_pair):
                xa = x8_cs[qc][:, c, :]
                x_dup = bass.AP(tensor=xa.tensor, offset=xa.offset,
                                ap=[xa.ap[0], [0, 2], xa.ap[-1]])
                nc.tensor.matmul(
                    ps, lhsT, x_dup,
                    start=(c == 0), stop=(c == KC - 1),
                    perf_mode=mybir.MatmulPerfMode.DoubleRow,
                )
        for ps, qc in zip(tiles, qc_pair):
            cols = slice(512 * qc, 512 * qc + 512)
            if name == "q":
                nc.vector.tensor_copy(qf8[:, pair, cols], ps)
            else:
                nc.vector.tensor_copy(kf8[:, pair, 0, cols], ps)
                nc.vector.tensor_sub(kf8[:, pair, 1, cols], ps,
                                     kf8[:, pair, 0, cols])

    def proj_v_st(st):
        ps = small_ps.tile([128, HL * HD], F32, tag="ps512", name="vps")
        for c in range(KC):
            nc.tensor.matmul(
                ps,
                xT_slice(c, 128 * st, 128),
                w_sb["v"][:, c, :],
                start=(c == 0), stop=(c == KC - 1),
            )
        nc.vector.tensor_copy(v_sb[:, st, :], ps)

    def alloc_head(h):
        e = epool.tile([128, E_TOT], BF16, tag="E", name=f"e{h}", bufs=3)
        zp = zpool.tile([128, KT, 3], F32, tag="zp", name=f"zp{h}")
        nc.vector.memset(zp, 0.0)
        return {"h": h, "e": e, "zp": zp}

    def score_bin(hs, bi):
        """One psum bin: DR matmuls + exp + diag masks + Z reduces."""
        h = hs["h"]
        pair, half = divmod(h, 2)
        pb = 64 * half
        wbin = BIN_W[bi]
        ps = sc_ps.tile([128, wbin], F32, tag="sc", name=f"sc{bi}")
        poff = 0
        for kt, lo, hi in BINS[bi]:
            w = hi - lo
            lhsT = kf8[pb:pb + 64, pair, :, 128 * kt:128 * kt + 128]
            c0 = poff
            while c0 < poff + w:
                c1 = min(poff + w, (c0 // 512 + 1) * 512)
                qg0 = 128 * kt + lo + (c0 - poff)
                qa = qf8[pb:pb + 64, pair, qg0:qg0 + (c1 - c0)]
                q_dup = bass.AP(tensor=qa.tensor, offset=qa.offset,
                                ap=[qa.ap[0], [0, 2], qa.ap[-1]])
                nc.tensor.matmul(
                    ps[:, c0:c1], lhsT, q_dup, start=True, stop=True,
                    perf_mode=mybir.MatmulPerfMode.DoubleRow,
                )
                c0 = c1
            poff += w
        eo = int(BIN_OFF[bi])
        akt = ACCUM_BINS.get(bi)
        if akt is not None:
            # single-row bin: Z via the activation accumulator; the invalid
            # diag half is gathered and subtracted via a negated reduce
            nc.scalar.activation(hs["e"][:, eo:eo + wbin], ps[:, 0:wbin],
                                 Exp, scale=SCALE_SC,
                                 accum_out=hs["zp"][:, akt, 0:1])
            diag = hs["e"][:, eo:eo + 128]
            scr = spool.tile([128, 128], BF16, tag="scr", name="scr", bufs=4)
            nc.gpsimd.affine_select(
                scr, diag, pattern=[[-1, 128]],
                compare_op=mybir.AluOpType.is_ge, fill=0.0,
                base=-1, channel_multiplier=1,
            )
            nc.vector.tensor_reduce(
                hs["zp"][:, akt, 1:2], scr,
                axis=mybir.AxisListType.X, op=mybir.AluOpType.add,
                negate=True,
            )
            nc.gpsimd.affine_select(
                diag, diag, pattern=[[1, 128]],
                compare_op=mybir.AluOpType.is_ge, fill=0.0,
                base=0, channel_multiplier=-1,
            )
            return
        nc.scalar.activation(hs["e"][:, eo:eo + wbin], ps[:, 0:wbin],
                             Exp, scale=SCALE_SC)
        # mixed bin: mask diags, then per-piece row sums (post-mask)
        off = eo
        for kt, lo, hi in BINS[bi]:
            w = hi - lo
            if lo == 0:
                diag = hs["e"][:, off:off + 128]
                nc.gpsimd.affine_select(
                    diag, diag, pattern=[[1, 128]],
                    compare_op=mybir.AluOpType.is_ge, fill=0.0,
                    base=0, channel_multiplier=-1,
                )
            slot = 2 if kt <= 3 else 0
            nc.vector.tensor_reduce(
                hs["zp"][:, kt, slot:slot + 1], hs["e"][:, off:off + w],
                axis=mybir.AxisListType.X, op=mybir.AluOpType.add,
            )
            off += w

    def z_v2(hs, k0, k1):
        """finalize Z for rows [k0, k1) and scale V rows by 1/Z."""
        h = hs["h"]
        n = k1 - k0
        zs = zpool.tile([128, n], F32, tag="zs", name="zs")
        nc.vector.tensor_reduce(zs, hs["zp"][:, k0:k1, :],
                                axis=mybir.AxisListType.X,
                                op=mybir.AluOpType.add)
        zi = zpool.tile([128, n], F32, tag="zi", name="zi")
        nc.vector.reciprocal(zi, zs)
        zia = zi[:, :]
        zi_bc = bass.AP(tensor=zia.tensor, offset=zia.offset,
                        ap=[zia.ap[0], zia.ap[1], [0, HD]])
        nc.gpsimd.tensor_mul(
            v2_sb[:, k0:k1, HD * h:HD * h + HD],
            v_sb[:, k0:k1, HD * h:HD * h + HD],
            zi_bc,
        )

    # ctx chains: one open psum tile per (pair, qc), fed incrementally
    ctx_state = {}

    def ctx_open(pair, qc):
        ctx_state[(pair, qc)] = small_ps.tile([128, 512], F32, tag="ps512",
                                              name=f"cp{pair}{qc}")

    def ctx_feed(pair, qc, kts, sta, stb):
        """emit ctx matmuls for key tiles `kts` into the open chain."""
        cp = ctx_state[(pair, qc)]
        n_kt = 4 * qc + 4
        for kt in kts:
            q0 = max(512 * qc, 128 * kt)
            q1 = 512 * qc + 512
            for half, hs in ((0, sta), (1, stb)):
                h = hs["h"]
                for lo, hi, eoff in SEG[kt]:
                    s0 = max(q0 - 128 * kt, lo)
                    s1 = min(q1 - 128 * kt, hi)
                    if s0 >= s1:
                        continue
                    rhs = hs["e"][:, eoff + s0 - lo:eoff + s1 - lo]
                    oc0 = 128 * kt + s0 - 512 * qc
                    nc.tensor.matmul(
                        cp[64 * half:64 * half + 64, oc0:oc0 + s1 - s0],
                        v2_sb[:, kt, HD * h:HD * h + HD],
                        rhs,
                        start=(kt == 0 and s0 == q0 - 128 * kt),
                        stop=(kt == n_kt - 1 and s1 == q1 - 128 * kt),
                        tile_position=(0, 64 * half),
                        skip_group_check=True,
                    )

    def ctx_close(pair, qc):
        cp = ctx_state.pop((pair, qc))
        ob = outp.tile([128, 512], F32, tag="ob", name="ob", bufs=2)
        nc.vector.tensor_copy(ob, cp)
        nc.sync.dma_start(
            out=out_ap[128 * pair:128 * pair + 128, 512 * qc:512 * qc + 512],
            in_=ob,
        )

    def ctx_all(pair, qc, sta, stb):
        ctx_open(pair, qc)
        ctx_feed(pair, qc, range(4 * qc + 4), sta, stb)
        ctx_close(pair, qc)

    # ---- emission (order = scheduling priority) ----
    st = [alloc_head(h) for h in range(HL)]

    # pair-0 projections interleaved with head-0 bins by availability
    proj_chain("q", 0, (3, 2))
    proj_chain("k", 0, (3, 2))
    for bi in AVAIL[0] + AVAIL[1]:
        score_bin(st[0], bi)
    proj_chain("q", 0, (1, 0))
    proj_chain("k", 0, (1, 0))
    for bi in AVAIL[2] + AVAIL[3]:
        score_bin(st[0], bi)
    # head-1 bins interleaved with pair-1 projections and v-chains;
    # p1 chains early so their DVE casts precede head-1's late Z work
    B_ORDER = [
        ('bin', 13), ('pq', (3, 2)), ('bin', 10), ('pk', (3, 2)),
        ('bin', 11), ('bin', 12), ('pq', (1, 0)), ('bin', 6),
        ('pk', (1, 0)), ('bin', 7), ('v', 0), ('bin', 8), ('v', 1),
        ('bin', 9), ('v', 2), ('bin', 0), ('v', 3), ('v', 4), ('bin', 1),
        ('v', 5), ('v', 6), ('bin', 2), ('v', 7), ('v', 8), ('bin', 3),
        ('v', 9), ('bin', 4), ('v', 10), ('bin', 5), ('v', 11),
    ]
    for kind, a in B_ORDER:
        if kind == 'bin':
            score_bin(st[1], a)
        elif kind == 'pq':
            proj_chain("q", 1, a)
        elif kind == 'pk':
            proj_chain("k", 1, a)
        else:
            proj_v_st(a)
    z_v2(st[0], 0, 12)
    z_v2(st[1], 0, 12)
    # head-2 bins paced against ctx pair-0 chains on PE
    score_bin(st[2], 13)
    ctx_all(0, 0, st[0], st[1])
    for bi in (10, 11, 12):
        score_bin(st[2], bi)
    ctx_all(0, 1, st[0], st[1])
    for bi in (6, 7):
        score_bin(st[2], bi)
    ctx_all(0, 2, st[0], st[1])
    for bi in (8, 9, 0, 1, 2, 3, 4, 5):
        score_bin(st[2], bi)
    z_v2(st[2], 0, 12)
    # boundary fillers: last v chains + deferred z tails while ACT drains h2
    for s in (12, 13, 14, 15):
        proj_v_st(s)
    z_v2(st[0], 12, KT)
    z_v2(st[1], 12, KT)
    z_v2(st[2], 12, KT)
    # head 3 progressive groups; ctx p0 qc3 + pair-1 chains fill PE slack
    for bi in H3_GROUPS[0]:
        score_bin(st[3], bi)
    ctx_all(0, 3, st[0], st[1])
    z_v2(st[3], 0, 4)
    ctx_all(1, 0, st[2], st[3])
    ctx_open(1, 1)
    ctx_feed(1, 1, range(0, 4), st[2], st[3])
    for bi in H3_GROUPS[1]:
        score_bin(st[3], bi)
    z_v2(st[3], 4, 8)
    ctx_feed(1, 1, range(4, 8), st[2], st[3])
    ctx_close(1, 1)
    ctx_open(1, 2)
    ctx_feed(1, 2, range(0, 8), st[2], st[3])
    for bi in H3_GROUPS[2]:
        score_bin(st[3], bi)
    z_v2(st[3], 8, 12)
    ctx_feed(1, 2, range(8, 12), st[2], st[3])
    ctx_close(1, 2)
    ctx_open(1, 3)
    ctx_feed(1, 3, range(0, 12), st[2], st[3])
    for bi in H3_GROUPS[3]:
        score_bin(st[3], bi)
    z_v2(st[3], 12, KT)
    ctx_feed(1, 3, range(12, 16), st[2], st[3])
    ctx_close(1, 3)


_PROG = None


def _build_program():
    global _PROG
    if _PROG is not None:
        return _PROG
    nc = bacc.Bacc("TRN2", target_bir_lowering=False, debug=False,
                   num_devices=NCORES)
    xT = nc.dram_tensor("xT", [D, S], BF16, kind="ExternalInput").ap()
    x8 = nc.dram_tensor("x8", [D, S], F8, kind="ExternalInput").ap()
    wq = nc.dram_tensor("wq", [D, 2, HL * HD], F8, kind="ExternalInput").ap()
    wk = nc.dram_tensor("wk", [D, 2, HL * HD], F8, kind="ExternalInput").ap()
    wv = nc.dram_tensor("wv", [D, HL * HD], BF16, kind="ExternalInput").ap()
    out = nc.dram_tensor("out", [HL * HD, S], F32, kind="ExternalOutput").ap()
    with tile.TileContext(nc) as tc:
        with ExitStack() as stack:
            _emit(stack, tc, out, xT, x8, wq, wk, wv)
    nc.compile()
    _PROG = nc
    return nc


def make_in_maps(x, Wq, Wk, Wv):
    bf = ml_dtypes.bfloat16
    f8 = ml_dtypes.float8_e4m3

    def w_hilo(W):
        Ws = np.asarray(W).astype(bf).astype(np.float32) * WS
        hi = Ws.astype(f8)
        lo = (Ws - hi.astype(np.float32)).astype(f8)
        return np.ascontiguousarray(np.stack([hi, lo], axis=1))  # [D, 2, n]

    in_maps = []
    for core in range(NCORES):
        b, g = divmod(core, NCORES // B)
        cols = slice(HL * HD * g, HL * HD * (g + 1))
        xTb = np.ascontiguousarray(np.asarray(x[b]).T)
        in_maps.append({
            "xT": xTb.astype(bf),
            "x8": xTb.astype(f8),
            "wq": w_hilo(np.asarray(Wq)[:, cols]),
            "wk": w_hilo(np.asarray(Wk)[:, cols]),
            "wv": np.ascontiguousarray(np.asarray(Wv)[:, cols]).astype(bf),
        })
    return in_maps


def assemble(results):
    out = np.empty((B, S, H * HD), np.float32)
    for core in range(NCORES):
        b, g = divmod(core, NCORES // B)
        out[b, :, HL * HD * g:HL * HD * (g + 1)] = results[core]["out"].T
    return out


def kernel(**inputs):
    nc = _build_program()
    in_maps = make_in_maps(inputs["x"], inputs["Wq"], inputs["Wk"], inputs["Wv"])
    res = run_bass_kernel_spmd(nc, in_maps, list(range(NCORES)))
    return assemble(res.results)


# revision 18
# speedup vs baseline: 1.1194x; 1.1194x over previous
"""Causal self-attention (softmax over the QUERY axis) for Trainium2, 8 cores.

Reference semantics (B=2, S=2048, D=1024, H=16, HD=64):
    q = x @ Wq; k = x @ Wk; v = x @ Wv          (per batch)
    s[b,h,q,k] = <q_bqh, k_bkh>;  mask k > q -> -inf
    w = softmax(s / sqrt(1024), axis=q)          # normalize over QUERY axis
    ctx[b,q,h,:] = sum_k w[b,h,q,k] * v[b,k,h,:]

Sharding: core c handles batch b = c // 4 and head group g = c % 4
(4 heads: 4g..4g+3).  Per core everything is done in a transposed
score layout S^T[k, q], which makes the query-axis softmax a FREE-AXIS
reduction, and the 1/Z[k] normalizer folds into V rows (no per-element
divide): ctx[q,d] = sum_k exp(s)/Z[k] * v[k,d] = sum_k exp(s) * (v[k,d]/Z[k]).

Device layouts (per core):
    xT  [1024, 2048] bf16 (host-transposed)  -> SBUF [128, 8, 2048]
    Wq/Wk/Wv column slices [1024, 256] bf16  -> SBUF [128, 8, 256]
    qT/kT  [128(2 heads x 64), 2 pairs, 2048] bf16 (projection output)
    v      [128(s in tile), 16 kt, 256(4 heads x 64)] bf16
    E      packed exp(scores^T): row kt occupies cols [off_kt, off_kt+2048-128kt)
    out    [256(4 heads x 64), 2048] f32 = ctx^T; host transposes back.
"""

import numpy as np
import ml_dtypes
from contextlib import ExitStack

import concourse.bass as bass
import concourse.tile as tile
from concourse import bacc, mybir
from concourse.bass_utils import run_bass_kernel_spmd

BF16 = mybir.dt.bfloat16
F32 = mybir.dt.float32

B, S, D, H, HD = 2, 2048, 1024, 16, 64
NCORES = 8
HL = 4                       # heads per core
KC = D // 128                # 8 contraction chunks
KT = S // 128                # 16 key tiles
QC = S // 512                # 4 query chunks of 512
SCALE = 1.0 / float(np.sqrt(np.float32(D)))   # 1/32

W_ROW = [S - 128 * kt for kt in range(KT)]          # valid width of E row kt
E_OFF = np.concatenate([[0], np.cumsum(W_ROW)]).astype(int)
E_TOT = int(E_OFF[-1])                              # 17408



def _emit(ctx: ExitStack, tc: tile.TileContext, out_ap, xT, wq, wk, wv):
    nc = tc.nc
    Exp = mybir.ActivationFunctionType.Exp

    consts = ctx.enter_context(tc.tile_pool(name="consts", bufs=1))
    qkp = ctx.enter_context(tc.tile_pool(name="qk", bufs=1))
    vp = ctx.enter_context(tc.tile_pool(name="v", bufs=1))
    epool = ctx.enter_context(tc.tile_pool(name="e", bufs=2))
    zpool = ctx.enter_context(tc.tile_pool(name="z", bufs=4))
    spool = ctx.enter_context(tc.tile_pool(name="scr", bufs=4))
    outp = ctx.enter_context(tc.tile_pool(name="outp", bufs=1))
    # scores rows: [128, 1536] = 3 banks x 2 bufs = 6 banks; projections and
    # ctx accumulations share one 2-slot [*, 512] pool (2 banks).
    sc_ps = ctx.enter_context(tc.tile_pool(name="sc_ps", bufs=2, space="PSUM"))
    small_ps = ctx.enter_context(tc.tile_pool(name="small_ps", bufs=2, space="PSUM"))

    # ---- loads: weights on the SP HWDGE ring, xT chunks on the ACT ring
    # (chunk 3 first: score rows are emitted descending) ----
    w_sb = {}
    for name, t in (("q", wq), ("k", wk), ("v", wv)):
        w_sb[name] = consts.tile([128, KC, HL * HD], BF16, tag=f"w{name}",
                                 name=f"w{name}_sb")
        nc.sync.dma_start(out=w_sb[name], in_=t.rearrange("(c p) n -> p c n", p=128))
    xT_r = xT.rearrange("(c p) s -> p c s", p=128)
    xT_cs = [None] * 4
    for sc in (3, 2, 1, 0):
        xT_cs[sc] = consts.tile([128, KC, 512], BF16, tag=f"xT{sc}",
                                name=f"xT{sc}_sb")
        nc.scalar.dma_start(out=xT_cs[sc],
                            in_=xT_r[:, :, 512 * sc:512 * sc + 512])

    def xT_slice(c, lo, w):
        sc, o = divmod(lo, 512)
        assert o + w <= 512
        return xT_cs[sc][:, c, o:o + w]

    qT_sb = qkp.tile([128, 2, S], BF16, tag="qT")
    kT_sb = qkp.tile([128, 2, S], BF16, tag="kT")
    v_sb = vp.tile([128, KT, HL * HD], BF16, tag="v")
    v2_sb = vp.tile([128, KT, HL * HD], BF16, tag="v2")
    out_sb = outp.tile([128, 2, S], F32, tag="out")

    def proj_chain(name, pair, qc):
        dst = qT_sb if name == "q" else kT_sb
        ps = small_ps.tile([128, 512], F32, tag="ps512", name="pps")
        for c in range(KC):
            nc.tensor.matmul(
                ps,
                w_sb[name][:, c, 128 * pair:128 * pair + 128],
                xT_cs[qc][:, c, :],
                start=(c == 0), stop=(c == KC - 1),
            )
        nc.vector.tensor_copy(dst[:, pair, 512 * qc:512 * qc + 512], ps)

    def proj_v():
        # v natural layout: out partitions = s-within-tile, cols = 4 heads x 64
        for st in range(KT):
            ps = small_ps.tile([128, HL * HD], F32, tag="ps512", name="pps")
            for c in range(KC):
                nc.tensor.matmul(
                    ps,
                    xT_slice(c, 128 * st, 128),
                    w_sb["v"][:, c, :],
                    start=(c == 0), stop=(c == KC - 1),
                )
            nc.vector.tensor_copy(v_sb[:, st, :], ps)

    def alloc_head(h):
        zp = zpool.tile([128, KT, 2], F32, tag="zp", name=f"zp{h}")
        inv = zpool.tile([128, KT], F32, tag="inv", name=f"inv{h}")
        nc.vector.memset(zp, 0.0)
        nc.vector.memset(inv, 0.0)
        return {"zp": zp, "inv": inv, "e": [None] * KT, "h": h}

    def score_row(st, kt):
        """scores^T row kt for head st['h']: matmuls + exp(+Z accum) + diag fix."""
        h = st["h"]
        pair, half = divmod(h, 2)
        pb = 64 * half
        q0k = 128 * kt
        W = S - q0k
        # rows 4..15 get a third slot so the next pair's score rows never
        # wait on ctx chains releasing E (rows 0..3 are too big to afford
        # a third copy, but they are also the last ones the next head
        # reaches, by which point the ctx chains have freed them).
        e_row = epool.tile([128, W], BF16, tag=f"E{kt}", name=f"e{kt}",
                           bufs=(3 if kt >= 4 else 2))
        st["e"][kt] = e_row
        lhsT = kT_sb[pb:pb + 64, pair, q0k:q0k + 128]   # [64, 128]
        tiles = [(q0k, min(W, 1536))]
        if W > 1536:
            tiles.append((q0k + 1536, W - 1536))
        dve_z = kt >= 8    # short rows: Z via DVE post-zero sum (ACT stays hot)
        for ti, (lo, w) in enumerate(tiles):
            ps = sc_ps.tile([128, w], F32, tag="sc", name="scps")
            c0 = 0
            while c0 < w:
                c1 = min(w, c0 + 512)
                nc.tensor.matmul(
                    ps[:, c0:c1],
                    lhsT,
                    qT_sb[pb:pb + 64, pair, lo + c0:lo + c1],
                    start=True, stop=True,
                )
                c0 = c1
            if dve_z:
                nc.scalar.activation(
                    e_row[:, lo - q0k:lo - q0k + w], ps[:, 0:w],
                    Exp, scale=SCALE,
                )
            else:
                nc.scalar.activation(
                    e_row[:, lo - q0k:lo - q0k + w], ps[:, 0:w],
                    Exp, scale=SCALE,
                    accum_out=st["zp"][:, kt, ti:ti + 1],
                )
        # diagonal block: cols [0, 128) hold q in [128kt, 128kt+128);
        # entries with q < k (j < p) are invalid.
        diag = e_row[:, 0:128]
        if not dve_z:
            # gather the invalid part (its sum is subtracted from Z);
            # is_lt is unimplemented in walrus codegen, so use is_ge with
            # negated affine coefficients (j < p <=> p - j - 1 >= 0).
            scr = spool.tile([128, 128], BF16, tag="scr", name="scr")
            nc.gpsimd.affine_select(
                scr, diag, pattern=[[-1, 128]],
                compare_op=mybir.AluOpType.is_ge, fill=0.0,
                base=-1, channel_multiplier=1,
            )
            nc.vector.tensor_reduce(
                st["inv"][:, kt:kt + 1], scr,
                axis=mybir.AxisListType.X, op=mybir.AluOpType.add,
            )
        nc.gpsimd.affine_select(
            diag, diag, pattern=[[1, 128]],
            compare_op=mybir.AluOpType.is_ge, fill=0.0,
            base=0, channel_multiplier=-1,
        )
        if dve_z:
            # post-zero row sum is exactly the valid Z contribution
            nc.vector.tensor_reduce(
                st["zp"][:, kt, 0:1], e_row[:, 0:W],
                axis=mybir.AxisListType.X, op=mybir.AluOpType.add,
            )

    def z_v2(st, k0, k1):
        """finalize Z for rows [k0, k1) and scale V rows by 1/Z."""
        h = st["h"]
        n = k1 - k0
        zs = zpool.tile([128, n], F32, tag="zs", name="zs")
        nc.vector.tensor_reduce(zs, st["zp"][:, k0:k1, :],
                                axis=mybir.AxisListType.X,
                                op=mybir.AluOpType.add)
        zv = zpool.tile([128, n], F32, tag="zv", name="zv")
        nc.vector.tensor_sub(zv, zs, st["inv"][:, k0:k1])
        zi = zpool.tile([128, n], F32, tag="zi", name="zi")
        nc.vector.reciprocal(zi, zv)
        zia = zi[:, :]
        zi_bc = bass.AP(tensor=zia.tensor, offset=zia.offset,
                        ap=[zia.ap[0], zia.ap[1], [0, HD]])
        nc.vector.tensor_mul(
            v2_sb[:, k0:k1, HD * h:HD * h + HD],
            v_sb[:, k0:k1, HD * h:HD * h + HD],
            zi_bc,
        )

    def ctx_chain(st, qc):
        """one solo ctx^T accumulation chain for (head, qc) + copy to out_sb."""
        h = st["h"]
        pair, half = divmod(h, 2)
        ps = small_ps.tile([64, 512], F32, tag="ps512", name="cps")
        n_kt = 4 * qc + 4
        for kt in range(n_kt):
            q0 = max(512 * qc, 128 * kt)
            w = 512 * qc + 512 - q0
            rhs = st["e"][kt][:, q0 - 128 * kt:q0 - 128 * kt + w]
            nc.tensor.matmul(
                ps[:, q0 - 512 * qc:512],
                v2_sb[:, kt, HD * h:HD * h + HD],
                rhs,
                start=(kt == 0), stop=(kt == n_kt - 1),
            )
        nc.vector.tensor_copy(
            out_sb[64 * half:64 * half + 64, pair, 512 * qc:512 * qc + 512], ps)

    def out_dma(pair, qc):
        nc.sync.dma_start(
            out=out_ap[128 * pair:128 * pair + 128, 512 * qc:512 * qc + 512],
            in_=out_sb[:, pair, 512 * qc:512 * qc + 512],
        )

    def ctx_pair_packed(sta, stb, qc):
        """col-packed ctx chains for a whole pair (heads sta, stb) at qc."""
        pair = sta["h"] // 2
        ps = small_ps.tile([128, 512], F32, tag="ps512", name="cpp")
        n_kt = 4 * qc + 4
        for kt in range(n_kt):
            q0 = max(512 * qc, 128 * kt)
            w = 512 * qc + 512 - q0
            for half, st in ((0, sta), (1, stb)):
                h = st["h"]
                rhs = st["e"][kt][:, q0 - 128 * kt:q0 - 128 * kt + w]
                nc.tensor.matmul(
                    ps[64 * half:64 * half + 64, q0 - 512 * qc:512],
                    v2_sb[:, kt, HD * h:HD * h + HD],
                    rhs,
                    start=(kt == 0), stop=(kt == n_kt - 1),
                    tile_position=(0, 64 * half),
                    skip_group_check=True,
                )
        nc.vector.tensor_copy(out_sb[:, pair, 512 * qc:512 * qc + 512], ps)

    # ---- emission (order = scheduling priority; heads' score rows always
    # outrank filler work so head transitions have no priority bubble) ----
    st0 = alloc_head(0)
    for qc in (3, 2, 1, 0):           # head 0 interleaved with its projections
        proj_chain("q", 0, qc)
        proj_chain("k", 0, qc)
        for kt in range(4 * qc + 3, 4 * qc - 1, -1):
            score_row(st0, kt)
    st1 = alloc_head(1)
    for kt in range(KT - 1, -1, -1):  # head 1 rows outrank all filler
        score_row(st1, kt)
    proj_v()                          # filler during heads 0-1 exp waits
    z_v2(st0, 0, KT)                  # (after proj_v: v_sb RAW order)
    z_v2(st1, 0, KT)
    for qc in (3, 2, 1, 0):           # pair-1 projections: filler
        proj_chain("q", 1, qc)
        proj_chain("k", 1, qc)
    st2 = alloc_head(2)
    for kt in range(KT - 1, -1, -1):  # E slots: rows 4-15 have a 3rd slot;
        score_row(st2, kt)            # rows 0-3 wait on the chain below
    ctx_pair_packed(st0, st1, 0)      # frees pair-0's E rows 0-3 early
    out_dma(0, 0)
    z_v2(st2, 0, KT)
    # head 3: ascending rows, per-group Z; overlaps head 2 on ACT since its
    # E slots are already free (3rd slot / chain-0 release)
    st3 = alloc_head(3)
    for g in range(4):
        for kt in range(4 * g, 4 * g + 4):
            score_row(st3, kt)
        z_v2(st3, 4 * g, 4 * g + 4)
        if g >= 1:                    # rest of pair-0 ctx: fills PE slack
            ctx_pair_packed(st0, st1, g)
            out_dma(0, g)
    for g in range(4):                # pair-1 ctx: packed, progressive
        ctx_pair_packed(st2, st3, g)
        out_dma(1, g)


_PROG = None


def _build_program():
    global _PROG
    if _PROG is not None:
        return _PROG
    nc = bacc.Bacc("TRN2", target_bir_lowering=False, debug=False,
                   num_devices=NCORES)
    xT = nc.dram_tensor("xT", [D, S], BF16, kind="ExternalInput").ap()
    wq = nc.dram_tensor("wq", [D, HL * HD], BF16, kind="ExternalInput").ap()
    wk = nc.dram_tensor("wk", [D, HL * HD], BF16, kind="ExternalInput").ap()
    wv = nc.dram_tensor("wv", [D, HL * HD], BF16, kind="ExternalInput").ap()
    out = nc.dram_tensor("out", [HL * HD, S], F32, kind="ExternalOutput").ap()
    with tile.TileContext(nc) as tc:
        with ExitStack() as stack:
            _emit(stack, tc, out, xT, wq, wk, wv)
    nc.compile()
    _PROG = nc
    return nc


def make_in_maps(x, Wq, Wk, Wv):
    bf = ml_dtypes.bfloat16
    in_maps = []
    for core in range(NCORES):
        b, g = divmod(core, NCORES // B)
        cols = slice(HL * HD * g, HL * HD * (g + 1))
        in_maps.append({
            "xT": np.ascontiguousarray(np.asarray(x[b]).T).astype(bf),
            "wq": np.ascontiguousarray(np.asarray(Wq)[:, cols]).astype(bf),
            "wk": np.ascontiguousarray(np.asarray(Wk)[:, cols]).astype(bf),
            "wv": np.ascontiguousarray(np.asarray(Wv)[:, cols]).astype(bf),
        })
    return in_maps


def assemble(results):
    out = np.empty((B, S, H * HD), np.float32)
    for core in range(NCORES):
        b, g = divmod(core, NCORES // B)
        out[b, :, HL * HD * g:HL * HD * (g + 1)] = results[core]["out"].T
    return out


def kernel(**inputs):
    nc = _build_program()
    in_maps = make_in_maps(inputs["x"], inputs["Wq"], inputs["Wk"], inputs["Wv"])
    res = run_bass_kernel_spmd(nc, in_maps, list(range(NCORES)))
    return assemble(res.results)

